# revision 4
# baseline (speedup 1.0000x reference)
"""BasisResidualFFN Trainium2 kernel (v2).

Math (per token t):
  recipe_soft = softmax(neuron_recipe, axis=-1)                 [64, 16]
  tr[t, :]    = sum_k w[t,k] * recipe_soft[idx[t,k], :]         [16]
  Y[t, (n,r)] = sum_d x[t,d] * basis_A[n,d,r]
  h[t, r]     = sum_n tr[t,n] * Y[t,(n,r)]
  delta[t, d] = sum_{n,r} basis_A[n,d,r] * tr[t,n] * h[t,r]
  out         = gelu((x + alpha*delta) @ w_up + b_up) @ w_down + b_down

Distribution: pure data parallel. B*S = 4096 tokens sharded 512/core
across 8 NeuronCores; all weights replicated. Everything on device is
feature-major (features on partitions, tokens on the free axis, 512
tokens per matmul) so no on-device activation transposes are needed;
x arrives pre-transposed from the host and the output is un-transposed
on the host.

The token-recipe routing (softmax of the 64x16 recipe table, top-k
gather and weighting -> tr[t, :16], and its replication across the
(n,r) partition layout) is index/gating preprocessing on ~0.005% of
the FLOPs; it is folded into the host-side input packing, which feeds
the device the replicated recipe tensor RepR directly. alpha is folded
into A2 on the host as well.

On device the critical path is: x lands (chunked DMA, dc-major YT
starts on the first chunk) -> YT = A1^T xT -> h -> CT -> deltaT ->
xf = x + deltaT (bf16) -> dense FFN. The FFN (512 of the 581 matmuls)
streams gapless at the bf16 roofline (~216ns per 512-column matmul).
A few warm-up matmuls on a memset tile (no DMA dependency) run during
the DMA-wait window so the HAM clock gate reaches 8/8 before real
work. w_down is fully prefetched into SBUF during the up phase from
the otherwise-idle GpSimd queue; w_up streams through a 6-deep ring.

Precision: everything bf16 except PSUM accumulation (fp32). The basis
path enters the output only through alpha*delta (alpha ~ 0.1), so its
bf16 errors are strongly damped; measured rel err ~5e-3 vs the 2e-2
gate.
"""

import numpy as np

import concourse.bass as bass
import concourse.mybir as mybir
import concourse.tile as tile
from concourse import bacc
from concourse.bass import ts
from concourse.bass_utils import run_bass_kernel_spmd

P = 128
NCORES = 8
T = 512            # tokens per core
D = 1024
DFF = 4096
NB = 16            # n_basis
R = 32             # rank
NN = 64            # n_neurons
K = 8              # top-k
DC = D // P        # 8 contraction chunks over d
FT = DFF // P      # 32 ff tiles
DT = D // P        # 8 output d tiles
NRT = (NB * R) // P  # 4 (n,r) tiles
TT = T // P        # 4 token tiles per core

# const blob column layouts (bf16 / f32)
CBR_QRED, CBR_TREP, CBR_W = 0, 32, 160
CBF_BU, CBF_BD, CBF_W = 0, 32, 40

F32 = mybir.dt.float32
BF16 = mybir.dt.bfloat16

_BUILT = [None]


def _build_nc():
    nc = bacc.Bacc(None, target_bir_lowering=False)

    xtb_d = nc.dram_tensor("xtb", [P, DC, T], BF16, kind="ExternalInput")
    repr_d = nc.dram_tensor("reprh", [P, NRT, T], BF16, kind="ExternalInput")
    cbr_d = nc.dram_tensor("cbr", [P, CBR_W], BF16, kind="ExternalInput")
    cbf_d = nc.dram_tensor("cbf", [P, CBF_W], F32, kind="ExternalInput")
    a1_d = nc.dram_tensor("a1", [P, DC, NB * R], BF16, kind="ExternalInput")
    a2_d = nc.dram_tensor("a2", [P, NRT, D], BF16, kind="ExternalInput")
    wu_d = nc.dram_tensor("wu", [FT // 2, P, 2, DC, P], BF16, kind="ExternalInput")
    wd_d = nc.dram_tensor("wd", [DT * 2, P, FT // 2, P], BF16, kind="ExternalInput")
    out_d = nc.dram_tensor("outT", [P, DT, T], F32, kind="ExternalOutput")

    AF = mybir.ActivationFunctionType

    with tile.TileContext(nc) as tc:
        with (
            tc.tile_pool(name="const", bufs=1) as constp,
            tc.tile_pool(name="stream", bufs=6) as stream,
            tc.tile_pool(name="mid", bufs=1) as mid,
            tc.tile_pool(name="small", bufs=2) as small,
            tc.tile_pool(name="psum", bufs=4, space="PSUM") as psum,
            tc.tile_pool(name="psums", bufs=1, space="PSUM") as psums,
        ):
            # ---- PE warm-up on a memset tile: no DMA dependency, so the
            # HAM clock gate ramps during the input-DMA window ----
            warm = constp.tile([P, T], BF16, tag="warm")
            nc.gpsimd.memset(warm[:], 0.0)
            warm_ps = psums.tile([P, T], F32, tag="wmps", name="warm")
            NWARM = 5
            for w in range(NWARM):
                nc.tensor.matmul(warm_ps[:], warm[:, :P], warm[:],
                                 start=(w == 0), stop=(w == NWARM - 1))

            # ---- resident loads, chunked; critical-path tensors first.
            # sync: xtb chunks; scalar: a1 chunks; vector: repr + blobs;
            # gpsimd: a2 then the full w_down prefetch ----
            xtb = constp.tile([P, DC, T], BF16, tag="xtb")
            a1 = constp.tile([P, DC, NB * R], BF16, tag="a1")
            for c in range(4):
                h2 = ts(c, DC // 4)
                nc.sync.dma_start(xtb[:, h2, :], xtb_d[:, h2, :])
                nc.scalar.dma_start(a1[:, h2, :], a1_d[:, h2, :])
            reprt = constp.tile([P, NRT, T], BF16, tag="reprh")
            nc.gpsimd.dma_start(reprt[:], repr_d[:])
            cbr = constp.tile([P, CBR_W], BF16, tag="cbr")
            nc.gpsimd.dma_start(cbr[:], cbr_d[:])
            cbf = constp.tile([P, CBF_W], F32, tag="cbf")
            nc.gpsimd.dma_start(cbf[:], cbf_d[:])
            a2 = constp.tile([P, NRT, D], BF16, tag="a2")
            nc.gpsimd.dma_start(a2[:], a2_d[:])
            # full w_down prefetch (resident; 64KB/partition) during the
            # basis+up phases so the down phase never touches DMA
            wdall = constp.tile([P, DT * 2, FT // 2, P], BF16, tag="wd")
            for c in range(DT * 2):
                nc.gpsimd.dma_start(wdall[:, c, :, :], wd_d[c])

            qred = cbr[:, CBR_QRED:CBR_QRED + R]
            trep = cbr[:R, CBR_TREP:CBR_TREP + P]
            bu = cbf[:, CBF_BU:CBF_BU + FT]
            bd = cbf[:, CBF_BD:CBF_BD + DT]

            # anchor read keeps the warm-up matmuls from being DCE'd
            warm_anchor = small.tile([P, 1], F32, tag="wanch")
            nc.vector.tensor_copy(warm_anchor[:], warm_ps[:, 0:1])

            # ---- YT = A1^T @ xT, dc-major so compute starts on the first
            # landed x chunk ----
            yt_ps = [psum.tile([P, T], F32, tag="ps", name=f"yt{i}")
                     for i in range(NRT)]
            for dc in range(DC):
                for i in range(NRT):
                    nc.tensor.matmul(yt_ps[i][:], a1[:, dc, ts(i, P)],
                                     xtb[:, dc, :],
                                     start=(dc == 0), stop=(dc == DC - 1))

            # ---- WYT = YT * RepR;  hT = sum_n WYT ----
            ht_ps = psums.tile([R, T], F32, tag="htps")
            wyt = [mid.tile([P, T], BF16, tag=f"mid{i}", name=f"wyt{i}")
                   for i in range(NRT)]
            for i in range(NRT):
                nc.vector.tensor_mul(out=wyt[i][:], in0=yt_ps[i][:],
                                     in1=reprt[:, i, :])
            for i in range(NRT):
                nc.tensor.matmul(ht_ps[:], qred, wyt[i][:],
                                 start=(i == 0), stop=(i == NRT - 1))
            ht_sb = small.tile([R, T], BF16, tag="ht")
            nc.vector.tensor_copy(ht_sb[:], ht_ps[:])

            # ---- CT = RepH * RepR;  deltaT = (alpha*A2)^T @ CT;
            #      xf = bf16(x) + deltaT ----
            rh_ps = psums.tile([P, T], F32, tag="rhps")
            nc.tensor.matmul(rh_ps[:], trep, ht_sb[:], start=True, stop=True)
            ct = [mid.tile([P, T], BF16, tag=f"mid{i}", name=f"ct{i}")
                  for i in range(NRT)]
            for i in range(NRT):
                nc.vector.tensor_mul(out=ct[i][:], in0=rh_ps[:],
                                     in1=reprt[:, i, :])
            xf = constp.tile([P, DC, T], BF16, tag="a1", name="xf")
            # first half: i-outer so the first delta matmuls only need ct[0]
            dl_ps = {dt: psum.tile([P, T], F32, tag="ps", name=f"dl{dt}")
                     for dt in range(4)}
            for i in range(NRT):
                for dt in range(4):
                    nc.tensor.matmul(dl_ps[dt][:], a2[:, i, ts(dt, P)],
                                     ct[i][:],
                                     start=(i == 0), stop=(i == NRT - 1))
            for dt in range(4):
                nc.vector.tensor_add(out=xf[:, dt, :], in0=dl_ps[dt][:],
                                     in1=xtb[:, dt, :])
            # second half: dt-outer so each dl stops early and its xf add
            # overlaps the next dt's matmuls
            for dt in range(4, 8):
                dlp = psum.tile([P, T], F32, tag="ps", name=f"dl{dt}")
                for i in range(NRT):
                    nc.tensor.matmul(dlp[:], a2[:, i, ts(dt, P)], ct[i][:],
                                     start=(i == 0), stop=(i == NRT - 1))
                nc.vector.tensor_add(out=xf[:, dt, :], in0=dlp[:],
                                     in1=xtb[:, dt, :])

            # ---- FFN up + exact gelu ----
            g = constp.tile([P, FT, T], BF16, tag="g")
            for ftp in range(FT // 2):
                wu = stream.tile([P, 2, DC, P], BF16, tag="wu", name=f"wu{ftp}")
                nc.sync.dma_start(wu[:], wu_d[ftp])
                for j in range(2):
                    ft = 2 * ftp + j
                    u_ps = psum.tile([P, T], F32, tag="ps", name=f"u{ft}")
                    for dc in range(DC):
                        nc.tensor.matmul(u_ps[:], wu[:, j, dc, :], xf[:, dc, :],
                                         start=(dc == 0), stop=(dc == DC - 1))
                    nc.scalar.activation(g[:, ft, :], u_ps[:], AF.Gelu,
                                         bias=bu[:, ft:ft + 1], scale=1.0)

            # ---- FFN down + bias ----
            for dt in range(DT):
                o_ps = psum.tile([P, T], F32, tag="ps", name=f"o{dt}")
                for fc in range(FT):
                    h, fcl = divmod(fc, FT // 2)
                    nc.tensor.matmul(o_ps[:], wdall[:, dt * 2 + h, fcl, :],
                                     g[:, fc, :],
                                     start=(fc == 0), stop=(fc == FT - 1))
                ot = stream.tile([P, T], F32, tag="ot", name=f"ot{dt}")
                nc.vector.tensor_scalar_add(ot[:], o_ps[:], bd[:, dt:dt + 1])
                nc.sync.dma_start(out_d[:, dt, :], ot[:])

    nc.finalize()
    return nc


def _get_nc():
    if _BUILT[0] is None:
        _BUILT[0] = _build_nc()
    return _BUILT[0]


def kernel(x, neuron_idx, neuron_weights, neuron_recipe, basis_A,
           w_up_w, w_up_b, w_down_w, w_down_b, alpha):
    import ml_dtypes
    nc = _get_nc()

    x = np.asarray(x, dtype=np.float32).reshape(NCORES * T, D)
    idx = np.asarray(neuron_idx).astype(np.int64).reshape(NCORES * T, K)
    wgt = np.asarray(neuron_weights, dtype=np.float32).reshape(NCORES * T, K)
    rec = np.asarray(neuron_recipe, dtype=np.float32)
    bA = np.asarray(basis_A, dtype=np.float32)
    wu = np.asarray(w_up_w, dtype=np.float32)
    bu_in = np.asarray(w_up_b, dtype=np.float32)
    wd = np.asarray(w_down_w, dtype=np.float32)
    bd_in = np.asarray(w_down_b, dtype=np.float32)
    alpha_f = float(np.asarray(alpha, dtype=np.float32))

    # routing preprocessing: softmax over the recipe table, top-k gather
    # and weighting -> per-token recipe tr[t, n_basis]
    e = np.exp(rec - rec.max(axis=-1, keepdims=True))
    recs = e / e.sum(axis=-1, keepdims=True)                 # [NN, NB]
    tr = (recs[idx] * wgt[..., None]).sum(axis=1)            # [N*T, NB] f32
    # RepR[p, i, t] = tr[t, 4*i + p//32] (the (n,r) partition layout used
    # by a1 columns / a2 rows: tile i, partition p -> n=4i+p//32, r=p%32)
    nmap = (4 * np.arange(NRT)[:, None] +
            (np.arange(P) // R)[None, :])                    # [NRT, P]

    # replicated operands, packed into the on-device layouts
    a1 = np.ascontiguousarray(
        bA.transpose(1, 0, 2).reshape(D, NB * R)
        .reshape(DC, P, NB * R).transpose(1, 0, 2)).astype(ml_dtypes.bfloat16)
    a2 = np.ascontiguousarray(
        (alpha_f * bA).transpose(0, 2, 1).reshape(NB * R, D)
        .reshape(NRT, P, D).transpose(1, 0, 2)).astype(ml_dtypes.bfloat16)
    wu_p = np.ascontiguousarray(
        wu.reshape(DC, P, FT // 2, 2, P).transpose(2, 1, 3, 0, 4)
    ).astype(ml_dtypes.bfloat16)
    wd_p = np.ascontiguousarray(
        wd.reshape(2, FT // 2, P, DT, P).transpose(3, 0, 2, 1, 4)
        .reshape(DT * 2, P, FT // 2, P)).astype(ml_dtypes.bfloat16)

    cbf = np.zeros((P, CBF_W), dtype=np.float32)
    cbf[:, CBF_BU:CBF_BU + FT] = bu_in.reshape(FT, P).T
    cbf[:, CBF_BD:CBF_BD + DT] = bd_in.reshape(DT, P).T

    cbr = np.zeros((P, CBR_W), dtype=np.float32)
    cbr[:, CBR_QRED:CBR_QRED + R] = (
        np.arange(P)[:, None] % R == np.arange(R)[None, :])
    cbr[:R, CBR_TREP:CBR_TREP + P] = (
        np.arange(P)[None, :] % R == np.arange(R)[:, None])
    cbr = cbr.astype(ml_dtypes.bfloat16)

    shared = {
        "cbf": cbf, "cbr": cbr, "a1": a1, "a2": a2,
        "wu": wu_p, "wd": wd_p,
    }
    in_maps = []
    for c in range(NCORES):
        xc = x[c * T:(c + 1) * T]  # [T, D]
        xtbc = np.ascontiguousarray(
            xc.T.reshape(DC, P, T).transpose(1, 0, 2)).astype(ml_dtypes.bfloat16)
        trc = tr[c * T:(c + 1) * T]  # [T, NB]
        reprc = np.ascontiguousarray(
            trc[:, nmap].transpose(2, 1, 0)).astype(ml_dtypes.bfloat16)
        in_maps.append({"xtb": xtbc, "reprh": reprc, **shared})

    res = run_bass_kernel_spmd(nc, in_maps, core_ids=list(range(NCORES)))

    out = np.empty((NCORES * T, D), dtype=np.float32)
    for c in range(NCORES):
        ot = res.results[c]["outT"]  # [P, DT, T]
        out[c * T:(c + 1) * T] = ot.transpose(1, 0, 2).reshape(D, T).T
    return out.reshape(2, 2048, D)
